# revision 1
# baseline (speedup 1.0000x reference)
"""ChebConv(K=2) + fc + log_softmax GNN kernel for 8 TRN2 NeuronCores.

Math (reference):
    deg[n]  = #edges with row==n ; dis = deg>0 ? 1/sqrt(max(deg,1)) : 0
    S[c,n]  = sum_{e: col=c,row=n} -dis[n]*dis[c]          (dense scatter matrix)
    h       = x@W0 + S@(x@W1) + b ; relu
    out     = log_softmax(h@Wf + bf, axis=1)

Key transform: (S@x)@W1 == S@(x@W1), so the per-edge gather/scatter runs on
[N,10] instead of [N,2048].  Work split over 8 cores by node rows (256 each):

  phase A: stream this core's x rows (2 MB, transposed layout from host) and
           matmul against [W0|W1] -> p^T [20,256] in PSUM      (fp32r, PE)
  comm:    AllGather of local p1 = (x@W1) rows (bf16 [256,10]) -> [2048,10]
  phase B: Tx1^T [10,256] = p1_all^T @ S^T[:,cols_this_core]   (bf16, PE)
           epilogue: h^T = p0^T + Tx1^T, relu(+b), @Wf (+bf), transpose,
           row-wise log_softmax, DMA out [256,10].

Host does index-only graph prep: degree histogram, dense S^T build (edge
multiplicities folded with dis scaling), and data layout/sharding.
"""

import sys

if "/opt/trn_rl_repo" not in sys.path:
    sys.path.insert(0, "/opt/trn_rl_repo")

import ml_dtypes
import numpy as np

import concourse.bass as bass  # noqa: F401  (import registers engine types)
import concourse.tile as tile
from concourse import bacc, mybir
from concourse.bass_utils import run_bass_kernel_spmd

N = 2048
FIN = 2048
G1 = 10
NCLS = 10
NCORES = 8
RPC = N // NCORES  # 256 rows per core
KT = FIN // 128  # 16 contraction tiles
BF16 = mybir.dt.bfloat16
F32 = mybir.dt.float32
F32R = mybir.dt.float32r
AF = mybir.ActivationFunctionType
ALU = mybir.AluOpType

_NC_CACHE = {}


def build_nc():
    nc = bacc.Bacc("TRN2", target_bir_lowering=False, debug=False, num_devices=NCORES)

    xt_d = nc.dram_tensor("xt", [128, KT, RPC], F32R, kind="ExternalInput")
    st_d = nc.dram_tensor("st", [128, KT, RPC], BF16, kind="ExternalInput")
    wc_d = nc.dram_tensor("wc", [128, KT, 20], F32R, kind="ExternalInput")
    wf_d = nc.dram_tensor("wf", [G1, NCLS], BF16, kind="ExternalInput")
    b_d = nc.dram_tensor("b", [G1, 1], F32, kind="ExternalInput")
    bf_d = nc.dram_tensor("bf", [NCLS, 1], F32, kind="ExternalInput")
    eye_d = nc.dram_tensor("eye", [G1, G1], F32, kind="ExternalInput")
    out_d = nc.dram_tensor("out", [RPC, NCLS], F32, kind="ExternalOutput")

    with (
        tile.TileContext(nc) as tc,
        tc.tile_pool(name="sb", bufs=1) as sb,
        tc.tile_pool(name="ps", bufs=1, space="PSUM") as psp,
        tc.tile_pool(name="dr", bufs=1, space="DRAM") as dr,
    ):
        # DRAM bounce buffers for the collective
        agin = dr.tile([RPC, G1], BF16, name="agin", tag="agin")
        agout = dr.tile([N, G1], BF16, addr_space="Shared", name="agout", tag="agout")
        # constants
        wc_sb = sb.tile([128, KT, 20], F32R, name="wc_sb", tag="wc_sb")
        wf_sb = sb.tile([G1, NCLS], BF16, name="wf_sb", tag="wf_sb")
        b_sb = sb.tile([G1, 1], F32, name="b_sb", tag="b_sb")
        bf_sb = sb.tile([NCLS, 1], F32, name="bf_sb", tag="bf_sb")
        eye_sb = sb.tile([G1, G1], F32, name="eye_sb", tag="eye_sb")
        nc.sync.dma_start(out=wc_sb[:], in_=wc_d.ap())
        nc.sync.dma_start(out=wf_sb[:], in_=wf_d.ap())
        nc.sync.dma_start(out=b_sb[:], in_=b_d.ap())
        nc.sync.dma_start(out=bf_sb[:], in_=bf_d.ap())
        nc.sync.dma_start(out=eye_sb[:], in_=eye_d.ap())

        # x (transposed layout) streamed in 4 chunks of 512 KB
        NXC = 4
        XCW = KT // NXC  # k-tiles per chunk
        xt_sb = []
        for j in range(NXC):
            t_ = sb.tile([128, XCW, RPC], F32R, name=f"xt_sb{j}", tag=f"xt_sb{j}")
            nc.sync.dma_start(out=t_[:], in_=xt_d.ap()[:, j * XCW : (j + 1) * XCW, :])
            xt_sb.append(t_)
        # S^T columns for this core, bf16, 2 chunks of 512 KB
        NSC = 2
        SCW = KT // NSC
        st_sb = []
        for j in range(NSC):
            t_ = sb.tile([128, SCW, RPC], BF16, name=f"st_sb{j}", tag=f"st_sb{j}")
            nc.sync.dma_start(out=t_[:], in_=st_d.ap()[:, j * SCW : (j + 1) * SCW, :])
            st_sb.append(t_)

        # phase A: p0^T/p1^T [10, 256] = W0^T/W1^T @ x_local^T
        ps_p0 = psp.tile([G1, RPC], F32, name="ps_p0", tag="ps_p0")
        ps_p1 = psp.tile([G1, RPC], F32, name="ps_p1", tag="ps_p1")
        for t in range(KT):
            rhs = xt_sb[t // XCW][:, t % XCW, :]
            nc.tensor.matmul(
                ps_p1[:],
                lhsT=wc_sb[:, t, G1 : 2 * G1],
                rhs=rhs,
                start=(t == 0),
                stop=(t == KT - 1),
            )
            nc.tensor.matmul(
                ps_p0[:],
                lhsT=wc_sb[:, t, 0:G1],
                rhs=rhs,
                start=(t == 0),
                stop=False,
            )

        # p1 rows -> node-major bf16, bounce to DRAM, AllGather
        p1T_sb = sb.tile([G1, RPC], F32, name="p1T_sb", tag="p1T_sb")
        nc.vector.tensor_copy(p1T_sb[:], ps_p1[:])
        for h in range(2):
            pt_ps = psp.tile([128, G1], F32, name=f"pt_ps{h}", tag=f"pt_ps{h}")
            nc.tensor.transpose(pt_ps[:], p1T_sb[:, h * 128 : (h + 1) * 128], eye_sb[:])
            pb = sb.tile([128, G1], BF16, name=f"p1b{h}", tag=f"p1b{h}")
            nc.vector.tensor_copy(pb[:], pt_ps[:])
            nc.sync.dma_start(out=agin[h * 128 : (h + 1) * 128, :], in_=pb[:])
        nc.gpsimd.collective_compute(
            "AllGather",
            ALU.bypass,
            replica_groups=[list(range(NCORES))],
            ins=[agin[:].opt()],
            outs=[agout[:].opt()],
        )
        p1all_sb = sb.tile([128, KT, G1], BF16, name="p1all_sb", tag="p1all_sb")
        nc.sync.dma_start(
            out=p1all_sb[:], in_=agout[:].rearrange("(t p) g -> p t g", p=128)
        )

        # phase B: accumulate Tx1^T = p1_all^T @ S^T[:, cols] on top of p0^T
        # (PSUM accumulation: h^T = p0^T + Tx1^T lands in ps_p0 for free)
        for t in range(KT):
            nc.tensor.matmul(
                ps_p0[:],
                lhsT=p1all_sb[:, t, :],
                rhs=st_sb[t // SCW][:, t % SCW, :],
                start=False,
                stop=(t == KT - 1),
            )

        # relu(h + b)
        hr_sb = sb.tile([G1, RPC], BF16, name="hr_sb", tag="hr_sb")
        nc.scalar.activation(hr_sb[:], ps_p0[:], AF.Relu, bias=b_sb[:])

        # logits^T [10, 256] = Wf^T @ h^T (+ bf)
        ps_lg = psp.tile([NCLS, RPC], F32, name="ps_lg", tag="ps_lg")
        nc.tensor.matmul(ps_lg[:], lhsT=wf_sb[:], rhs=hr_sb[:], start=True, stop=True)
        lgT_sb = sb.tile([NCLS, RPC], F32, name="lgT_sb", tag="lgT_sb")
        nc.vector.tensor_scalar_add(lgT_sb[:], ps_lg[:], bf_sb[:])

        # transpose logits, row-wise log_softmax, write out
        for h in range(2):
            lg_ps = psp.tile([128, NCLS], F32, name=f"lg_ps{h}", tag=f"lg_ps{h}")
            nc.tensor.transpose(lg_ps[:], lgT_sb[:, h * 128 : (h + 1) * 128], eye_sb[:])
            nmax = sb.tile([128, 1], F32, name=f"nmax{h}", tag=f"nmax{h}")
            nc.vector.tensor_reduce(
                nmax[:], lg_ps[:], axis=mybir.AxisListType.X, op=ALU.max, negate=True
            )
            e_sb = sb.tile([128, NCLS], F32, name=f"e_sb{h}", tag=f"e_sb{h}")
            ssum = sb.tile([128, 1], F32, name=f"ssum{h}", tag=f"ssum{h}")
            nc.scalar.activation(e_sb[:], lg_ps[:], AF.Exp, bias=nmax[:], accum_out=ssum[:])
            lsum = sb.tile([128, 1], F32, name=f"lsum{h}", tag=f"lsum{h}")
            nc.scalar.activation(lsum[:], ssum[:], AF.Ln)
            o_sb = sb.tile([128, NCLS], F32, name=f"o_sb{h}", tag=f"o_sb{h}")
            nc.vector.tensor_scalar(
                o_sb[:], lg_ps[:], nmax[:], lsum[:], op0=ALU.add, op1=ALU.subtract
            )
            nc.sync.dma_start(out=out_d.ap()[h * 128 : (h + 1) * 128, :], in_=o_sb[:])

    nc.compile()
    return nc


def prep_inputs(x, edge_index, W0, W1, b, Wf, bf):
    """Host-side sharding/layout. Returns per-core in_maps."""
    x = np.asarray(x, np.float32)
    edge_index = np.asarray(edge_index)
    W0 = np.asarray(W0, np.float32)
    W1 = np.asarray(W1, np.float32)
    b = np.asarray(b, np.float32)
    Wf = np.asarray(Wf, np.float32)
    bf = np.asarray(bf, np.float32)

    row = edge_index[0].astype(np.int64)
    col = edge_index[1].astype(np.int64)
    deg = np.bincount(row, minlength=N).astype(np.float32)
    dis = np.where(deg > 0, 1.0 / np.sqrt(np.maximum(deg, 1.0)), 0.0).astype(np.float32)

    # dense S^T with multiplicities and dis scaling folded in
    mult = np.bincount(row * N + col, minlength=N * N).astype(np.float32).reshape(N, N)
    st_full = (-(dis[:, None] * dis[None, :]) * mult).astype(ml_dtypes.bfloat16)
    st3 = st_full.reshape(KT, 128, N)

    wc = np.concatenate([W0, W1], axis=1)  # [2048, 20]
    wc_arr = np.ascontiguousarray(wc.reshape(KT, 128, 20).transpose(1, 0, 2))
    wf_arr = np.ascontiguousarray(Wf.astype(ml_dtypes.bfloat16))
    b_arr = np.ascontiguousarray(b.reshape(G1, 1))
    bf_arr = np.ascontiguousarray(bf.reshape(NCLS, 1))
    eye_arr = np.eye(G1, dtype=np.float32)

    in_maps = []
    for c in range(NCORES):
        r0 = c * RPC
        xs = x[r0 : r0 + RPC, :]  # [256, 2048]
        xt = np.ascontiguousarray(xs.reshape(RPC, KT, 128).transpose(2, 1, 0))
        st = np.ascontiguousarray(st3[:, :, r0 : r0 + RPC].transpose(1, 0, 2))
        in_maps.append(
            {
                "xt": xt,
                "st": st,
                "wc": wc_arr,
                "wf": wf_arr,
                "b": b_arr,
                "bf": bf_arr,
                "eye": eye_arr,
            }
        )
    return in_maps


def kernel(x, edge_index, W0, W1, b, Wf, bf, _trace=False, _trace_kwargs=None):
    in_maps = prep_inputs(x, edge_index, W0, W1, b, Wf, bf)
    if "nc" not in _NC_CACHE:
        _NC_CACHE["nc"] = build_nc()
    nc = _NC_CACHE["nc"]
    res = run_bass_kernel_spmd(
        nc,
        in_maps,
        core_ids=list(range(NCORES)),
        trace=_trace,
        **(_trace_kwargs or {}),
    )
    out = np.concatenate([m["out"] for m in res.results], axis=0).astype(np.float32)
    if _trace:
        kernel.last_results = res
    return out



# revision 6
# speedup vs baseline: 1.4864x; 1.4864x over previous
"""ChebConv(K=2) + fc + log_softmax GNN kernel for 8 TRN2 NeuronCores.

Math (reference):
    deg[n]  = #edges with row==n ; dis = deg>0 ? 1/sqrt(max(deg,1)) : 0
    S[c,n]  = sum_{e: col=c,row=n} -dis[n]*dis[c]          (dense scatter matrix)
    h       = x@W0 + S@(x@W1) + b ; relu
    out     = log_softmax(h@Wf + bf, axis=1)

Key transforms:
  * (S@x)@W1 == S@(x@W1): per-edge work runs on [N,10] not [N,2048].
  * NO collective. A profile of the AllGather variant showed ~45 us of the
    95 us runtime spent in the CC barrier (core launch skew + ncfw floor).
    Instead every core reads the FULL x (bf16, 8 MB @ ~358 GB/s ~= 22 us)
    and computes p1 = x@W1 for all 2048 nodes locally -> zero cross-core
    sync, each core's span is just its own DMA-bound pipeline.
  * Node axis is ROLLED per core by 256*core so "own" rows are always
    cols 0:256 -> one SPMD program, no per-core slicing.
  * Epilogue stays in [10, n] transposed layout: per-node softmax sums via
    a ones-vector matmul, -log(sum) broadcast back via a k=1 matmul. No
    max-subtraction (|logits| ~ few units, exp is safe in f32).

Per-core schedule (node dim split in 4 quarters of 512):
  qA   : 16 matmuls  [wc_t 128x20]^T @ [x_t 128x512] -> psum bank q [20,512]
         (rows 0:10 = x@W0, rows 10:20 = p1 = x@W1)
  p1n  : psum rows 10:20 -> sbuf bf16 [10,512], DVE 32x32 block transposes
         -> node-major p1n [128, j, 10]
  B    : 16 matmuls p1n_j^T @ st_j -> Tx1 [10,256] (accumulate in psum)
  epi  : h = relu(p0_own + Tx1 + b); lg = Wf^T@h; e=exp(lg+bf);
         ssum = ones^T@e; lg -= ln(ssum) (k=1 matmul); out = lg + bf.

Host does index-only graph prep: degree histogram, dense S^T build (edge
multiplicities folded with dis scaling), per-core roll + layout + bf16.
"""

import sys

if "/opt/trn_rl_repo" not in sys.path:
    sys.path.insert(0, "/opt/trn_rl_repo")

import ml_dtypes
import numpy as np

import concourse.bass as bass  # noqa: F401  (import registers engine types)
import concourse.tile as tile
from concourse import bacc, mybir
from concourse.bass_utils import run_bass_kernel_spmd

N = 2048
FIN = 2048
G1 = 10
NCLS = 10
NCORES = 8
RPC = N // NCORES  # 256 own rows per core
KT = FIN // 128  # 16 contraction tiles
NQ = 4  # node-dim quarters
QW = N // NQ  # 512 nodes per quarter
BF16 = mybir.dt.bfloat16
F32 = mybir.dt.float32
AF = mybir.ActivationFunctionType
ALU = mybir.AluOpType

_NC_CACHE = {}


def build_nc():
    nc = bacc.Bacc("TRN2", target_bir_lowering=False, debug=False, num_devices=NCORES)

    # x^T, bf16, rolled so own nodes are first: [part, quarter, ktile, node]
    xt_d = nc.dram_tensor("xt", [128, NQ, KT, QW], BF16, kind="ExternalInput")
    # S^T slice (rolled rows), node-block-major: [part, block, own-col]
    st_d = nc.dram_tensor("st", [128, KT, RPC], BF16, kind="ExternalInput")
    wc_d = nc.dram_tensor("wc", [128, KT, 2 * G1], BF16, kind="ExternalInput")
    wf_d = nc.dram_tensor("wf", [G1, NCLS], BF16, kind="ExternalInput")
    b_d = nc.dram_tensor("b", [G1, 1], F32, kind="ExternalInput")
    bf_d = nc.dram_tensor("bf", [NCLS, 1], F32, kind="ExternalInput")
    ones_d = nc.dram_tensor("ones", [NCLS, 1], BF16, kind="ExternalInput")
    nones_d = nc.dram_tensor("nones", [1, NCLS], BF16, kind="ExternalInput")
    out_d = nc.dram_tensor("out", [NCLS, RPC], F32, kind="ExternalOutput")

    with (
        tile.TileContext(nc) as tc,
        tc.tile_pool(name="sb", bufs=1) as sb,
        tc.tile_pool(name="ps", bufs=1, space="PSUM") as psp,
    ):
        # constants
        wc_sb = sb.tile([128, KT, 2 * G1], BF16, name="wc_sb", tag="wc_sb")
        wf_sb = sb.tile([G1, NCLS], BF16, name="wf_sb", tag="wf_sb")
        b_sb = sb.tile([G1, 1], F32, name="b_sb", tag="b_sb")
        bf_sb = sb.tile([NCLS, 1], F32, name="bf_sb", tag="bf_sb")
        ones_sb = sb.tile([NCLS, 1], BF16, name="ones_sb", tag="ones_sb")
        nones_sb = sb.tile([1, NCLS], BF16, name="nones_sb", tag="nones_sb")
        nc.sync.dma_start(out=wc_sb[:], in_=wc_d.ap())
        nc.sync.dma_start(out=wf_sb[:], in_=wf_d.ap())
        nc.sync.dma_start(out=b_sb[:], in_=b_d.ap())
        nc.sync.dma_start(out=bf_sb[:], in_=bf_d.ap())
        nc.sync.dma_start(out=ones_sb[:], in_=ones_d.ap())
        nc.sync.dma_start(out=nones_sb[:], in_=nones_d.ap())

        # x quarters, each as 2 half-chunks of 1 MB for DMA/compute pipelining
        KH = KT // 2
        x_sb = [[None, None] for _ in range(NQ)]
        st_sb = [None, None]

        def dma_x(q, h):
            t_ = sb.tile([128, KH, QW], BF16, name=f"x{q}{h}", tag=f"x{q}{h}")
            nc.sync.dma_start(out=t_[:], in_=xt_d.ap()[:, q, h * KH : (h + 1) * KH, :])
            x_sb[q][h] = t_

        def dma_st(h):
            t_ = sb.tile([128, KH, RPC], BF16, name=f"st{h}", tag=f"st{h}")
            nc.sync.dma_start(out=t_[:], in_=st_d.ap()[:, h * KH : (h + 1) * KH, :])
            st_sb[h] = t_

        dma_x(0, 0)
        dma_x(0, 1)
        dma_st(0)
        dma_x(1, 0)
        dma_x(1, 1)
        dma_x(2, 0)
        dma_x(2, 1)
        dma_st(1)
        dma_x(3, 0)
        dma_x(3, 1)

        # [p0|p1]^T staging in SBUF (PSUM reads must start at partition 0, so
        # the whole 20-partition bank is copied); rows 20:32 zeroed once so
        # the 32x32 block transposes never read uninitialized SBUF
        cp = sb.tile([32, NQ, QW], BF16, name="cp", tag="cp")
        nc.vector.memset(cp[:], 0.0)
        # node-major [node-part, block, g]: cols 0:10 = p0 (unused), 10:20 = p1
        p1n = sb.tile([128, KT, 32], BF16, name="p1n", tag="p1n")

        banks = [
            psp.tile([2 * G1, QW], F32, name=f"bank{q}", tag=f"bank{q}")
            for q in range(NQ)
        ]
        ps_tx = psp.tile([G1, RPC], F32, name="ps_tx", tag="ps_tx")

        for q in range(NQ):
            # phase A: [p0|p1]^T quarter = wc^T @ x^T
            for t in range(KT):
                nc.tensor.matmul(
                    banks[q][:],
                    lhsT=wc_sb[:, t, :],
                    rhs=x_sb[q][t // KH][:, t % KH, :],
                    start=(t == 0),
                    stop=(t == KT - 1),
                )
            # bank -> bf16 staging, then 32x32 block transposes to node-major
            nc.vector.tensor_copy(cp[0 : 2 * G1, q, :], banks[q][:])
            for j in range(4 * q, 4 * (q + 1)):  # 128-node blocks
                for i in range(4):  # 32-node subblocks
                    l_ = 4 * (j - 4 * q) + i
                    nc.vector.transpose(
                        p1n[32 * i : 32 * (i + 1), j, :],
                        cp[:, q, 32 * l_ : 32 * (l_ + 1)],
                    )
            # phase B: Tx1 += p1n_j^T @ st_j for this quarter's node blocks
            for j in range(4 * q, 4 * (q + 1)):
                nc.tensor.matmul(
                    ps_tx[:],
                    lhsT=p1n[:, j, G1 : 2 * G1],
                    rhs=st_sb[j // (KT // 2)][:, j % (KT // 2), :],
                    start=(j == 0),
                    stop=(j == KT - 1),
                )

        # epilogue, all in [10, 256] transposed layout
        hsum = sb.tile([G1, RPC], F32, name="hsum", tag="hsum")
        nc.vector.tensor_add(hsum[:], ps_tx[:], cp[0:G1, 0, 0:RPC])
        hr = sb.tile([G1, RPC], BF16, name="hr", tag="hr")
        nc.scalar.activation(hr[:], hsum[:], AF.Relu, bias=b_sb[:])

        ps_lg = psp.tile([NCLS, RPC], F32, name="ps_lg", tag="ps_lg")
        nc.tensor.matmul(ps_lg[:], lhsT=wf_sb[:], rhs=hr[:], start=True, stop=True)
        e_sb = sb.tile([NCLS, RPC], BF16, name="e_sb", tag="e_sb")
        nc.scalar.activation(e_sb[:], ps_lg[:], AF.Exp, bias=bf_sb[:])

        ps_sum = psp.tile([1, RPC], F32, name="ps_sum", tag="ps_sum")
        nc.tensor.matmul(ps_sum[:], lhsT=ones_sb[:], rhs=e_sb[:], start=True, stop=True)
        lsum = sb.tile([1, RPC], BF16, name="lsum", tag="lsum")
        nc.scalar.activation(lsum[:], ps_sum[:], AF.Ln)

        # out = logits - ln(sum): -ln(sum) broadcast via k=1 outer-product
        # matmul, then the Wf matmul re-issued on top (both cheap), so every
        # psum read above hits a closed accumulation group.
        ps_lg2 = psp.tile([NCLS, RPC], F32, name="ps_lg2", tag="ps_lg2")
        nc.tensor.matmul(
            ps_lg2[:], lhsT=nones_sb[:], rhs=lsum[:], start=True, stop=False
        )
        nc.tensor.matmul(ps_lg2[:], lhsT=wf_sb[:], rhs=hr[:], start=False, stop=True)
        outT = sb.tile([NCLS, RPC], F32, name="outT", tag="outT")
        nc.vector.tensor_scalar_add(outT[:], ps_lg2[:], bf_sb[:])
        nc.sync.dma_start(out=out_d.ap(), in_=outT[:])

    nc.compile()
    return nc


def prep_inputs(x, edge_index, W0, W1, b, Wf, bf):
    """Host-side sharding/layout. Returns per-core in_maps."""
    x = np.asarray(x, np.float32)
    edge_index = np.asarray(edge_index)
    W0 = np.asarray(W0, np.float32)
    W1 = np.asarray(W1, np.float32)
    b = np.asarray(b, np.float32)
    Wf = np.asarray(Wf, np.float32)
    bf = np.asarray(bf, np.float32)

    row = edge_index[0].astype(np.int64)
    col = edge_index[1].astype(np.int64)
    deg = np.bincount(row, minlength=N).astype(np.float32)
    dis = np.where(deg > 0, 1.0 / np.sqrt(np.maximum(deg, 1.0)), 0.0).astype(np.float32)

    # dense S^T [src, dst] with multiplicities and dis scaling folded in
    mult = np.bincount(row * N + col, minlength=N * N).astype(np.float32).reshape(N, N)
    st_full = (-(dis[:, None] * dis[None, :]) * mult).astype(ml_dtypes.bfloat16)

    xb = x.astype(ml_dtypes.bfloat16)
    wc = np.concatenate([W0, W1], axis=1).astype(ml_dtypes.bfloat16)  # [2048, 20]
    wc_arr = np.ascontiguousarray(wc.reshape(KT, 128, 2 * G1).transpose(1, 0, 2))
    wf_arr = np.ascontiguousarray(Wf.astype(ml_dtypes.bfloat16))
    b_arr = np.ascontiguousarray(b.reshape(G1, 1))
    bf_arr = np.ascontiguousarray(bf.reshape(NCLS, 1))
    ones_arr = np.ones((NCLS, 1), ml_dtypes.bfloat16)
    nones_arr = np.full((1, NCLS), -1.0, ml_dtypes.bfloat16)

    in_maps = []
    for c in range(NCORES):
        r0 = c * RPC
        xr = np.roll(xb, -r0, axis=0)  # rolled nodes: own rows first
        # xt[p, q, t, n] = xr[512q + n, 128t + p]
        xt = np.ascontiguousarray(
            xr.T.reshape(KT, 128, NQ, QW).transpose(1, 2, 0, 3)
        )
        sr = np.roll(st_full, -r0, axis=0)[:, r0 : r0 + RPC]  # [2048, 256]
        st = np.ascontiguousarray(sr.reshape(KT, 128, RPC).transpose(1, 0, 2))
        in_maps.append(
            {
                "xt": xt,
                "st": st,
                "wc": wc_arr,
                "wf": wf_arr,
                "b": b_arr,
                "bf": bf_arr,
                "ones": ones_arr,
                "nones": nones_arr,
            }
        )
    return in_maps


def kernel(x, edge_index, W0, W1, b, Wf, bf, _trace=False, _trace_kwargs=None):
    in_maps = prep_inputs(x, edge_index, W0, W1, b, Wf, bf)
    if "nc" not in _NC_CACHE:
        _NC_CACHE["nc"] = build_nc()
    nc = _NC_CACHE["nc"]
    res = run_bass_kernel_spmd(
        nc,
        in_maps,
        core_ids=list(range(NCORES)),
        trace=_trace,
        **(_trace_kwargs or {}),
    )
    out = np.concatenate(
        [np.asarray(m["out"], np.float32).T for m in res.results], axis=0
    )
    if _trace:
        kernel.last_results = res
    return out


# revision 11
# speedup vs baseline: 1.8052x; 1.2145x over previous
"""ChebConv(K=2) + fc + log_softmax GNN kernel for 8 TRN2 NeuronCores.

Math (reference):
    deg[n]  = #edges with row==n ; dis = deg>0 ? 1/sqrt(max(deg,1)) : 0
    S[c,n]  = sum_{e: col=c,row=n} -dis[n]*dis[c]          (dense scatter matrix)
    h       = x@W0 + S@(x@W1) + b ; relu
    out     = log_softmax(h@Wf + bf, axis=1)

Key transforms:
  * (S@x)@W1 == S@(x@W1): per-edge work runs on [N,10] not [N,2048].
  * NO collective. A profile of the AllGather variant showed ~45 us of the
    95 us runtime spent in the CC barrier (core launch skew + ncfw floor).
    Instead every core reads the FULL x (bf16, 8 MB @ ~430 GB/s ~= 19 us)
    and computes p1 = x@W1 for all 2048 nodes locally -> zero cross-core
    sync, each core's span is just its own DMA-bound pipeline.
  * Node axis is ROLLED per core by 256*core so "own" rows are always
    cols 0:256 -> one SPMD program, no per-core slicing.
  * ALL constants ride in ONE f32 DMA (separate small DMAs each pay ~2 us
    completion latency serialized at the front of the HWDGE queue).
  * Epilogue stays in [10, n] transposed layout: per-node softmax sums via
    a ones-vector matmul, -log(sum) broadcast back via a k=1 matmul. No
    max-subtraction (|logits| ~ few units, exp is safe in f32). relu+bias
    on DVE (tensor_scalar) keeps ScalarE down to Exp/Ln, whose ACT tables
    are pre-warmed during the initial DMA wait.

Per-core schedule (node dim split in 4 quarters of 512):
  qA   : 16 matmuls  [wc_t 128x20]^T @ [x_t 128x512] -> psum bank q [20,512]
         (rows 0:10 = x@W0, rows 10:20 = p1 = x@W1)
  p1n  : bank -> sbuf bf16 copy, 4 strided 4-block 32x32 DVE transposes
         -> node-major p1n [128, j, g]
  B    : 4 matmuls p1n_j^T @ st_j -> Tx1 [10,256] (psum accumulate)
  epi  : h = relu(p0_own + Tx1 + b); lg = Wf^T@h; e=exp(lg+bf);
         ssum = ones^T@e; out = lg - ln(ssum) + bf.

Host does index-only graph prep: degree histogram, dense S^T build (edge
multiplicities folded with dis scaling), per-core roll + layout + bf16.
"""

import sys

if "/opt/trn_rl_repo" not in sys.path:
    sys.path.insert(0, "/opt/trn_rl_repo")

import ml_dtypes
import numpy as np

import concourse.bass as bass  # noqa: F401  (import registers engine types)
import concourse.tile as tile
from concourse import bacc, mybir
from concourse.bass_utils import run_bass_kernel_spmd

N = 2048
FIN = 2048
G1 = 10
NCLS = 10
NCORES = 8
RPC = N // NCORES  # 256 own rows per core
KT = FIN // 128  # 16 contraction tiles
NQ = 4  # node-dim quarters
QW = N // NQ  # 512 nodes per quarter
CW = 20 * KT  # flattened wc columns
BF16 = mybir.dt.bfloat16
F32 = mybir.dt.float32
AF = mybir.ActivationFunctionType
ALU = mybir.AluOpType

_NC_CACHE = {}


def build_nc():
    nc = bacc.Bacc("TRN2", target_bir_lowering=False, debug=False, num_devices=NCORES)

    # x^T, bf16, rolled so own nodes are first: [part, quarter, ktile, node]
    xt_d = nc.dram_tensor("xt", [128, NQ, KT, QW], BF16, kind="ExternalInput")
    # S^T slice (rolled rows), node-block-major: [part, block, own-col]
    st_d = nc.dram_tensor("st", [128, KT, RPC], BF16, kind="ExternalInput")
    # all constants in one f32 tensor: [:, 0:320] wc, [0:10, 320:330] Wf,
    # [0:10, 330] b, [0:10, 331] bf
    cst_d = nc.dram_tensor("cst", [128, CW + 12], F32, kind="ExternalInput")
    out_d = nc.dram_tensor("out", [NCLS, RPC], F32, kind="ExternalOutput")

    with (
        tile.TileContext(nc) as tc,
        tc.tile_pool(name="sb", bufs=1) as sb,
        tc.tile_pool(name="ps", bufs=1, space="PSUM") as psp,
    ):
        cst = sb.tile([128, CW + 12], F32, name="cst", tag="cst")
        nc.sync.dma_start(out=cst[:], in_=cst_d.ap())

        # x quarters, each as 2 half-chunks of 1 MB for DMA/compute pipelining
        KH = KT // 2
        x_sb = [[None, None] for _ in range(NQ)]
        st_sb = [None, None]

        def dma_x(q, h):
            t_ = sb.tile([128, KH, QW], BF16, name=f"x{q}{h}", tag=f"x{q}{h}")
            nc.sync.dma_start(out=t_[:], in_=xt_d.ap()[:, q, h * KH : (h + 1) * KH, :])
            x_sb[q][h] = t_

        def dma_st(h):
            t_ = sb.tile([128, KH, RPC], BF16, name=f"st{h}", tag=f"st{h}")
            nc.sync.dma_start(out=t_[:], in_=st_d.ap()[:, h * KH : (h + 1) * KH, :])
            st_sb[h] = t_

        dma_x(0, 0)
        dma_x(0, 1)
        dma_st(0)
        dma_x(1, 0)
        dma_x(1, 1)
        dma_x(2, 0)
        dma_x(2, 1)
        dma_st(1)
        dma_x(3, 0)
        dma_x(3, 1)

        # on-device const prep (overlaps the x stream)
        wc_sb = sb.tile([128, CW], BF16, name="wc_sb", tag="wc_sb")
        nc.vector.tensor_copy(wc_sb[:], cst[:, 0:CW])
        wf_sb = sb.tile([G1, NCLS], BF16, name="wf_sb", tag="wf_sb")
        nc.vector.tensor_copy(wf_sb[:], cst[0:G1, CW : CW + 10])
        b_ap = cst[0:G1, CW + 10 : CW + 11]
        bf_ap = cst[0:G1, CW + 11 : CW + 12]
        ones_sb = sb.tile([NCLS, 1], BF16, name="ones_sb", tag="ones_sb")
        nc.vector.memset(ones_sb[:], 1.0)
        nones_sb = sb.tile([1, NCLS], BF16, name="nones_sb", tag="nones_sb")
        nc.vector.memset(nones_sb[:], -1.0)

        # pre-warm ScalarE activation tables for Exp/Ln during the DMA wait
        warm = sb.tile([1, 3], F32, name="warm", tag="warm")
        nc.vector.memset(warm[:], 1.0)
        nc.scalar.activation(warm[0:1, 1:2], warm[0:1, 0:1], AF.Exp)
        nc.scalar.activation(warm[0:1, 2:3], warm[0:1, 0:1], AF.Ln)

        # [p0|p1]^T staging in SBUF (PSUM reads must start at partition 0, so
        # the whole 20-partition bank is copied); rows 20:32 zeroed once so
        # the 32x32 block transposes never read uninitialized SBUF.
        # free layout [i, j_local, c] so each subblock-i transpose input is a
        # contiguous [32, 128] strip (block l_global = 4*j_local + i)
        cp = sb.tile([32, NQ, 4, 4, 32], BF16, name="cp", tag="cp")
        nc.vector.memset(cp[:], 0.0)
        # node-major [node-part, block, g]: cols 0:10 = p0 (unused), 10:20 = p1
        p1n = sb.tile([128, KT, 32], BF16, name="p1n", tag="p1n")

        # psum bank free layout [j_local, i, c]: flat node order within quarter
        banks = [
            psp.tile([2 * G1, 4, 4, 32], F32, name=f"bank{q}", tag=f"bank{q}")
            for q in range(NQ)
        ]
        ps_tx = psp.tile([G1, RPC], F32, name="ps_tx", tag="ps_tx")

        for q in range(NQ):
            # phase A: [p0|p1]^T quarter = wc^T @ x^T
            for t in range(KT):
                nc.tensor.matmul(
                    banks[q][:],
                    lhsT=wc_sb[:, 20 * t : 20 * (t + 1)],
                    rhs=x_sb[q][t // KH][:, t % KH, :],
                    start=(t == 0),
                    stop=(t == KT - 1),
                )
            # bank -> bf16 staging (permuted to i-major), then one 4-block
            # 32x32 transpose call per 32-partition output group
            nc.vector.tensor_copy(
                cp[0 : 2 * G1, q], banks[q][:].rearrange("p j i c -> p i j c")
            )
            for i in range(4):
                nc.vector.transpose(
                    p1n[32 * i : 32 * (i + 1), 4 * q : 4 * (q + 1), :],
                    cp[:, q, i],
                )
            # phase B: Tx1 += p1n_j^T @ st_j for this quarter's node blocks
            for j in range(4 * q, 4 * (q + 1)):
                nc.tensor.matmul(
                    ps_tx[:],
                    lhsT=p1n[:, j, G1 : 2 * G1],
                    rhs=st_sb[j // KH][:, j % KH, :],
                    start=(j == 0),
                    stop=(j == KT - 1),
                )

        # epilogue, all in [10, 256] transposed layout
        # p0_own from the SBUF staging copy (a second PSUM read is illegal in
        # tensor_tensor), rearranged back to node-ascending order
        hsum = sb.tile([G1, RPC], F32, name="hsum", tag="hsum")
        p0_ap = cp[0:G1, 0, :, 0:2, :].rearrange("p i j c -> p j i c")
        nc.vector.tensor_add(hsum[:], ps_tx[:], p0_ap)
        hr = sb.tile([G1, RPC], BF16, name="hr", tag="hr")
        nc.vector.tensor_scalar(hr[:], hsum[:], b_ap, 0.0, op0=ALU.add, op1=ALU.max)

        ps_lg = psp.tile([NCLS, RPC], F32, name="ps_lg", tag="ps_lg")
        nc.tensor.matmul(ps_lg[:], lhsT=wf_sb[:], rhs=hr[:], start=True, stop=True)
        e_sb = sb.tile([NCLS, RPC], BF16, name="e_sb", tag="e_sb")
        nc.scalar.activation(e_sb[:], ps_lg[:], AF.Exp, bias=bf_ap)

        ps_sum = psp.tile([1, RPC], F32, name="ps_sum", tag="ps_sum")
        nc.tensor.matmul(ps_sum[:], lhsT=ones_sb[:], rhs=e_sb[:], start=True, stop=True)
        lsum = sb.tile([1, RPC], BF16, name="lsum", tag="lsum")
        nc.scalar.activation(lsum[:], ps_sum[:], AF.Ln)

        # out = logits - ln(sum): -ln(sum) broadcast via k=1 outer-product
        # matmul, then the Wf matmul re-issued on top (both cheap), so every
        # psum read above hits a closed accumulation group.
        ps_lg2 = psp.tile([NCLS, RPC], F32, name="ps_lg2", tag="ps_lg2")
        nc.tensor.matmul(
            ps_lg2[:], lhsT=nones_sb[:], rhs=lsum[:], start=True, stop=False
        )
        nc.tensor.matmul(ps_lg2[:], lhsT=wf_sb[:], rhs=hr[:], start=False, stop=True)
        outT = sb.tile([NCLS, RPC], F32, name="outT", tag="outT")
        nc.vector.tensor_scalar_add(outT[:], ps_lg2[:], bf_ap)
        nc.sync.dma_start(out=out_d.ap(), in_=outT[:])

    nc.compile()
    return nc


def prep_inputs(x, edge_index, W0, W1, b, Wf, bf):
    """Host-side sharding/layout. Returns per-core in_maps."""
    x = np.asarray(x, np.float32)
    edge_index = np.asarray(edge_index)
    W0 = np.asarray(W0, np.float32)
    W1 = np.asarray(W1, np.float32)
    b = np.asarray(b, np.float32)
    Wf = np.asarray(Wf, np.float32)
    bf = np.asarray(bf, np.float32)

    row = edge_index[0].astype(np.int64)
    col = edge_index[1].astype(np.int64)
    deg = np.bincount(row, minlength=N).astype(np.float32)
    dis = np.where(deg > 0, 1.0 / np.sqrt(np.maximum(deg, 1.0)), 0.0).astype(np.float32)

    # dense S^T [src, dst] with multiplicities and dis scaling folded in
    mult = np.bincount(row * N + col, minlength=N * N).astype(np.float32).reshape(N, N)
    st_full = (-(dis[:, None] * dis[None, :]) * mult).astype(ml_dtypes.bfloat16)

    xb = x.astype(ml_dtypes.bfloat16)
    wc = np.concatenate([W0, W1], axis=1)  # [2048, 20] f32
    cst = np.zeros((128, CW + 12), np.float32)
    cst[:, 0:CW] = wc.reshape(KT, 128, 2 * G1).transpose(1, 0, 2).reshape(128, CW)
    cst[0:G1, CW : CW + 10] = Wf
    cst[0:G1, CW + 10] = b
    cst[0:G1, CW + 11] = bf

    in_maps = []
    for c in range(NCORES):
        r0 = c * RPC
        xr = np.roll(xb, -r0, axis=0)  # rolled nodes: own rows first
        # xt[p, q, t, n] = xr[512q + n, 128t + p]
        xt = np.ascontiguousarray(
            xr.T.reshape(KT, 128, NQ, QW).transpose(1, 2, 0, 3)
        )
        sr = np.roll(st_full, -r0, axis=0)[:, r0 : r0 + RPC]  # [2048, 256]
        st = np.ascontiguousarray(sr.reshape(KT, 128, RPC).transpose(1, 0, 2))
        in_maps.append({"xt": xt, "st": st, "cst": cst})
    return in_maps


def kernel(x, edge_index, W0, W1, b, Wf, bf, _trace=False, _trace_kwargs=None):
    in_maps = prep_inputs(x, edge_index, W0, W1, b, Wf, bf)
    if "nc" not in _NC_CACHE:
        _NC_CACHE["nc"] = build_nc()
    nc = _NC_CACHE["nc"]
    res = run_bass_kernel_spmd(
        nc,
        in_maps,
        core_ids=list(range(NCORES)),
        trace=_trace,
        **(_trace_kwargs or {}),
    )
    out = np.concatenate(
        [np.asarray(m["out"], np.float32).T for m in res.results], axis=0
    )
    if _trace:
        kernel.last_results = res
    return out
